# revision 1
# baseline (speedup 1.0000x reference)
"""Multi-head attention (dense transformer block) for 8 Trainium2 NeuronCores.

Problem: x [4, 2048, 1024] f32, w_qkv [3072, 1024], w_out [1024, 1024]
  qkv = x @ w_qkv.T ; split q,k,v ; 16 heads x 64 dims
  out = softmax(q k^T / 8) v ; y = out @ w_out.T

Sharding: 8 shards = (batch b in 0..3) x (head-half hh in 0..1).
Each core handles one batch and 8 heads end-to-end: QKV projection
column-split, attention for its 8 heads, out-projection row-split ->
partial y. Host sums the two partial y's per batch. No collectives.

Kernel structure: one flat software pipeline over (pair, tq, kt) —

    v | qk(0) | B(0)+qk(1) | B(1)+qk(2)+C(1) | B(2)+qk(3) | B(3)+C(3)

  - v: value projection -> vaug bf16 [ktok, head, 65] with a ones column.
  - qk(p): q^T,k^T [128, tok] for pair p; only kT + qT(tq0) run before
    B(0), the rest (and all of qk(p+1), C(p)) drain from a filler queue
    at ~2 matmuls per kt INSIDE the attention loop, filling the PE's
    es-wait bubbles so ScalarE's exp stream never starves.
  - B(p): attention. Scores computed transposed per head S^T[ktok, qtok]
    with the two heads PAIRED via PE row-tiling (K=64 at partitions
    0/64) into adjacent PSUM banks; one ScalarE exp ACTIVATE [128, 1024]
    per k-tile covers both heads with the 1/8 scale folded in (softmax
    max-subtraction skipped; scores are O(+-6)). AV matmuls in bf16 with
    M=65: the ones column makes PSUM row 64 the softmax denominators.
    The AV matmul lags scores/exp by one k-tile ACROSS tq and pair
    boundaries (scores of kt+1 issue before the AV flush of kt).
    Softmax epilogue (emitted at the NEXT tq's first iteration): DVE
    copies drain av out of PSUM in ~1.3us so the PE can reuse the bank
    immediately (no HAM re-throttle), then DVE reciprocal_approx_fast
    (denominator staged at base_partition 0 — the custom-DVE op
    mis-executes at nonzero base), GpSimd partition-broadcast, DVE mult.
  - C(p): per-pair out-projection (K=128) into a full-size SBUF y
    accumulator (pair 1 copies, pair 3 adds); one big sync-DMA per rep
    writes y. No gpsimd DMA accum (IRAM ucode thrash vs broadcast).
  - Cross-rep pipelining (multi-rep timing builds): at B(3,tq1) the next
    rep's x DMAs fire and its value-projection groups join the filler
    queue (vaug double-buffered, v copies on DVE), so the next rep
    starts with v already materialized.
"""

import numpy as np

B = 4
NT = 2048          # tokens per batch
E = 1024           # embed dim
H = 16             # heads
DH = 64            # head dim
HD = 512           # head dims per core (8 heads)
N_CORES = 8
SCALE = DH ** -0.5
P = 128

_cache = {}


def _build(rep=1, ablate=(), mmdt="f32r", loop=False):
    import concourse.mybir as mybir
    import concourse.tile as tile
    from concourse import bacc
    from contextlib import ExitStack
    from collections import deque

    # dtype scheme: f32r/bf16/fp16 uniform; "mix" = fp16 q/k path + bf16 soft path
    f32 = mybir.dt.float32
    _qk = {"f32r": mybir.dt.float32r, "bf16": mybir.dt.bfloat16,
           "fp16": mybir.dt.float16, "mix": mybir.dt.float16}
    _soft = {"f32r": mybir.dt.bfloat16, "bf16": mybir.dt.bfloat16,
             "fp16": mybir.dt.float16, "mix": mybir.dt.bfloat16}
    f32r = _qk[mmdt]          # q/k-side matmul dtype (x, wq, wk, wv, qT, kT)
    bf16 = _soft[mmdt]        # softmax/out-side dtype (es, vaug, outT, woT)
    in_dt = {"f32r": f32, "bf16": mybir.dt.bfloat16,
             "fp16": mybir.dt.float16, "mix": mybir.dt.float16}[mmdt]
    wo_dt = {"f32r": f32, "bf16": mybir.dt.bfloat16,
             "fp16": mybir.dt.float16, "mix": mybir.dt.bfloat16}[mmdt]
    Exp = mybir.ActivationFunctionType.Exp
    Add = mybir.AluOpType.add

    nc = bacc.Bacc("TRN2", target_bir_lowering=False, debug=False,
                   enable_asserts=False, num_devices=N_CORES)

    xT_ap = nc.dram_tensor("xT", [E, NT], in_dt, kind="ExternalInput").ap()
    wqT_ap = nc.dram_tensor("wqT", [E, HD], in_dt, kind="ExternalInput").ap()
    wkT_ap = nc.dram_tensor("wkT", [E, HD], in_dt, kind="ExternalInput").ap()
    wvT_ap = nc.dram_tensor("wvT", [E, HD], in_dt, kind="ExternalInput").ap()
    woT_ap = nc.dram_tensor("woT", [HD, E], wo_dt, kind="ExternalInput").ap()
    y_ap = nc.dram_tensor("y", [NT, E], f32, kind="ExternalOutput").ap()

    KE = E // P        # 8 contraction tiles over embed
    MQ = HD // P       # 4 partition tiles over head dims = head pairs
    TQ = NT // 512     # 4 query chunks of 512
    TT = NT // P       # 16 token tiles of 128

    from concourse.tile_rust import add_dep_helper

    with tile.TileContext(nc) as tc, ExitStack() as ctx:
        per = ctx.enter_context(tc.tile_pool(name="per", bufs=1))
        qk_pool = ctx.enter_context(tc.tile_pool(name="qk", bufs=2))
        outT_pool = ctx.enter_context(tc.tile_pool(name="ot", bufs=2))
        es_pool = ctx.enter_context(tc.tile_pool(name="es", bufs=3))
        nrm_pool = ctx.enter_context(tc.tile_pool(name="nrm", bufs=2))
        bcs_pool = ctx.enter_context(tc.tile_pool(name="bcs", bufs=2))
        stg_pool = ctx.enter_context(tc.tile_pool(name="stg", bufs=4))
        xT_pool = ctx.enter_context(tc.tile_pool(name="xTp", bufs=1))
        vaug_pool = ctx.enter_context(tc.tile_pool(name="vap", bufs=2))
        psS = ctx.enter_context(tc.tile_pool(name="psS", bufs=2, space="PSUM"))
        psAV = ctx.enter_context(tc.tile_pool(name="psAV", bufs=2, space="PSUM"))
        psM = ctx.enter_context(tc.tile_pool(name="psM", bufs=2, space="PSUM"))

        # rep-invariant weights (wv first: the value projection runs first)
        wv = per.tile([P, KE, HD], f32r, tag="wv")
        nc.scalar.dma_start(wv[:], wvT_ap.rearrange("(o p) m -> p o m", p=P).bitcast(f32r))
        wq = per.tile([P, KE, HD], f32r, tag="wq")
        nc.scalar.dma_start(wq[:], wqT_ap.rearrange("(o p) m -> p o m", p=P).bitcast(f32r))
        wk = per.tile([P, KE, HD], f32r, tag="wk")
        nc.scalar.dma_start(wk[:], wkT_ap.rearrange("(o p) m -> p o m", p=P).bitcast(f32r))
        woT = per.tile([P, MQ, E], bf16, tag="woT")
        nc.scalar.dma_start(woT[:], woT_ap.rearrange("(o p) e -> p o e", p=P).bitcast(bf16))
        # full y accumulator in SBUF [tok%128, tok//128, embed]: pair 1
        # writes, pair 3 adds, one big DMA per rep writes it out. Avoids
        # gpsimd-triggered DRAM accum DMAs (IRAM ucode thrash against
        # partition_broadcast) and halves y DRAM traffic.
        ysb_full = per.tile([P, TT, E], f32, tag="ysbf")

        # Tile does not order DMAs by DRAM range: chain each y region's
        # write/accumulate DMAs explicitly (across pairs and reps).
        y_prev_dma = {}

        def emit_xt():
            xTs = []
            xT_src = xT_ap.rearrange("(o p) t -> p o t", p=P).bitcast(f32r)
            for ke in range(KE):
                xk = xT_pool.tile([P, NT], f32r, tag=f"xT{ke}", name=f"xT{ke}")
                nc.sync.dma_start(xk[:], xT_src[:, ke, :])
                xTs.append(xk)
            return xTs

        def alloc_vaug():
            # double-buffered (bufs=2) so the next rep's value projection can
            # fill fresh buffers while this rep's B(3) still reads the old
            vaug_g = [vaug_pool.tile([P, 4, 8, DH + 1], bf16, tag=f"vaug{g}",
                                     name=f"vaug{g}") for g in range(TT // 4)]
            for g in range(TT // 4):
                nc.vector.memset(vaug_g[g][:, :, :, DH:DH + 1], 1.0)
            return [vaug_g[tt // 4][:, tt % 4] for tt in range(TT)]

        def emit_body(xTs, vaugs, first, has_next):
            # PE filler queue: qk-projection and out-projection matmuls are
            # interleaved into the attention kt loop (~2 matmuls per kt) so
            # ScalarE's exp stream never waits behind a burst of projection
            # work, and the PE's es-wait bubbles get filled.
            filler_q = deque()

            def filler_step(n):
                while n > 0 and filler_q:
                    try:
                        next(filler_q[0])
                        n -= 1
                    except StopIteration:
                        filler_q.popleft()

            def gen_qk_group(mq, dst, w, tq, rot=0):
                ps = psM.tile([P, 512], f32, tag="m")
                for i in range(KE):
                    ke = (i + rot) % KE
                    nc.tensor.matmul(ps[:], w[:, ke, mq * P:(mq + 1) * P],
                                     xTs[ke][:, tq * 512:(tq + 1) * 512],
                                     start=(i == 0), stop=(i == KE - 1))
                    if i < KE - 1:
                        yield
                nc.vector.tensor_copy(dst[:, tq * 512:(tq + 1) * 512], ps[:])
                yield

            def emit_qk_group(mq, dst, w, tq, rot=0):
                for _ in gen_qk_group(mq, dst, w, tq, rot):
                    pass

            def gen_outproj(pair, outT_a, outT_b, tq):
                # y[tq tokens] (+)= outT.T @ woT for this pair of head-pairs
                for tt in range(tq * 4, tq * 4 + 4):
                    for ec in range(E // 512):
                        esl = slice(ec * 512, (ec + 1) * 512)
                        ps = psM.tile([P, 512], f32, tag="m")
                        nc.tensor.matmul(ps[:], outT_a[:, tt * P:(tt + 1) * P],
                                         woT[:, pair - 1, esl],
                                         start=True, stop=False)
                        yield
                        nc.tensor.matmul(ps[:], outT_b[:, tt * P:(tt + 1) * P],
                                         woT[:, pair, esl],
                                         start=False, stop=True)
                        if pair == 1:
                            nc.vector.tensor_copy(ysb_full[:, tt, esl], ps[:])
                        else:
                            nc.vector.tensor_tensor(ysb_full[:, tt, esl], ps[:],
                                                    ysb_full[:, tt, esl], Add)
                        yield

            def alloc_qk(mq):
                qT = qk_pool.tile([P, NT], f32r, tag="qTp", name=f"qT{mq}")
                kT = qk_pool.tile([P, NT], f32r, tag="kTp", name=f"kT{mq}")
                return qT, kT

            def qk_groups(mq, qT, kT):
                for dst, w in ((kT, wk), (qT, wq)):
                    for tq in range(TQ):
                        yield (mq, dst, w, tq)

            def gen_v_group(vxTs, vvaugs, tt):
                ps = psM.tile([P, HD], f32, tag="m")
                for i in range(KE):
                    ke = (i + tt) % KE
                    nc.tensor.matmul(ps[:], vxTs[ke][:, tt * P:(tt + 1) * P],
                                     wv[:, ke, :], start=(i == 0),
                                     stop=(i == KE - 1))
                    if i < KE - 1:
                        yield
                # copy on Vector, not Scalar: as a cross-rep filler this
                # lands in the middle of B(3)'s exp stream otherwise
                nc.vector.tensor_copy(vvaugs[tt][:, :, 0:DH],
                                      ps[:].rearrange("p (h d) -> p h d", h=8))
                yield

            def emit_epilogue(pair, tq, avs, outT):
                # Copy both av tiles out of PSUM first: the bank is freed
                # after ~1.3us of copies instead of after the full reciprocal
                # -> broadcast -> multiply chain (~5us), which stalled the
                # PE each tq and re-throttled it to 1.2 GHz (HAM). The denom
                # row gets its own base-partition-0 tile: the custom-DVE
                # reciprocal_approx_fast mis-executes at base_partition != 0.
                qsl = slice(tq * 512, (tq + 1) * 512)
                stgs = []
                for av in avs:
                    stg = stg_pool.tile([DH, 512], bf16, tag="stg")
                    dn = nrm_pool.tile([1, 512], f32, tag="dn")
                    nc.vector.tensor_copy(stg[:], av[0:DH, :])
                    nc.vector.tensor_copy(dn[:], av[DH:DH + 1, :])
                    stgs.append((stg, dn))
                for j, (stg, dn) in enumerate(stgs):
                    recip = nrm_pool.tile([1, 512], f32, tag="recip")
                    nc.vector.reciprocal_approx_fast(recip[:], dn[:])
                    bcs = bcs_pool.tile([DH, 512], f32, tag="bcs")
                    nc.gpsimd.partition_broadcast(bcs[:], recip[:])
                    nc.vector.tensor_tensor(
                        outT[j * DH:(j + 1) * DH, qsl],
                        stg[:], bcs[:], mybir.AluOpType.mult)

            # One flat software pipeline over (pair, tq, kt), with the AV
            # matmul lagging scores/exp by one k-tile ACROSS tq and pair
            # boundaries: scores(kt+1) always issue before the AV flush of
            # kt, so ScalarE's exp stream never has a boundary bubble. Each
            # tq's normalize epilogue is emitted at the next tq's first
            # iteration (right after its final AV lands). qk(p+1) and
            # C(p odd) matmuls drain from filler_q at ~2 per kt.
            #   v | qk(0) | B(0)+qk(1) | B(1)+qk(2)+C(1) | ... | B(3)+C(3)
            if first:
                for tt in range(TT):
                    for _ in gen_v_group(xTs, vaugs, tt):
                        pass
            qT, kT = alloc_qk(0)
            groups = list(qk_groups(0, qT, kT))
            # B(0,tq0) consumes kT group g from kt=4g and qT(tq0) from kt0:
            # emit kT(0,1) + qT(tq0) up front, drain the rest at 3/kt
            # during tq0.
            for gi in (0, 1, 4):
                emit_qk_group(*groups[gi], rot=gi)
            for gi in (2, 3, 5, 6, 7):
                filler_q.append(gen_qk_group(*groups[gi], rot=gi))

            nxt = None
            qks = {0: (qT, kT)}
            outTs = {}
            pending = None      # (avs, pair, kt, es)
            ep_pending = None   # (pair, tq, avs)
            avs = None
            qk_iter = iter(())
            NG = MQ * TQ * TT
            for g in range(NG + 1):
                pair, r = divmod(g, TQ * TT)
                tq, kt = divmod(r, TT)
                last = g == NG
                if not last and kt == 0:
                    if tq == 0:
                        qT, kT = qks[pair]
                        outTs[pair] = outT_pool.tile(
                            [P, NT], bf16, tag="outT", name=f"outT{pair}")
                        if pair + 1 < MQ:
                            qks[pair + 1] = alloc_qk(pair + 1)
                            qk_iter = qk_groups(pair + 1, *qks[pair + 1])
                        else:
                            qk_iter = iter(())
                    for _ in range(2):
                        qg = next(qk_iter, None)
                        if qg is not None:
                            filler_q.append(gen_qk_group(*qg))
                if not last:
                    qsl = slice(tq * 512, (tq + 1) * 512)
                    ksl = slice(kt * P, (kt + 1) * P)
                    sps = psS.tile([P, 2, 512], f32, tag="s")
                    nc.tensor.matmul(sps[:, 0, :], kT[0:DH, ksl],
                                     qT[0:DH, qsl], start=True, stop=True)
                    nc.tensor.matmul(sps[:, 1, :], kT[DH:P, ksl],
                                     qT[DH:P, qsl], start=True, stop=True)
                    es = es_pool.tile([P, 2, 512], bf16, tag="es")
                    nc.scalar.activation(es[:], sps[:], Exp, scale=SCALE)
                    # fillers BEFORE the av flush: the av matmul stalls on
                    # exp(kt-1)'s semaphore, and the in-order PE queue would
                    # hold the next scores (which gate exp(kt+1)) behind it.
                    # With fillers here the post-stall path to the next
                    # scores is just the two av matmuls.
                    filler_step(3 if g < TT else 2)
                if pending is not None:
                    pavs, ppair, pkt, pes = pending
                    nc.tensor.matmul(pavs[0][:], vaugs[pkt][:, 2 * ppair, :],
                                     pes[:, 0, :],
                                     start=(pkt == 0), stop=(pkt == TT - 1))
                    nc.tensor.matmul(pavs[1][:], vaugs[pkt][:, 2 * ppair + 1, :],
                                     pes[:, 1, :],
                                     start=(pkt == 0), stop=(pkt == TT - 1))
                    pending = None
                if kt == 0 and ep_pending is not None:
                    ep_pair, ep_tq, ep_avs = ep_pending
                    emit_epilogue(ep_pair, ep_tq, ep_avs, outTs[ep_pair])
                    ep_pending = None
                    if ep_pair % 2 == 1:
                        filler_q.append(gen_outproj(
                            ep_pair, outTs[ep_pair - 1], outTs[ep_pair], ep_tq))
                if kt == 0 and tq == 0 and pair == MQ - 1 and has_next:
                    # cross-rep pipeline, stage 1: the next rep's x DMAs
                    # fire at B(3,tq0) (their xT WAR deps -- qk(3) reads --
                    # resolved during B(2)). The 8 chunk DMAs serialize at
                    # ~3us each on the sync queue, so they need a full tq
                    # of lead before the v fillers start consuming them.
                    nxt_xTs = emit_xt()
                if kt == 0 and tq == 1 and pair == MQ - 1 and has_next:
                    # stage 2: the next rep's value-projection groups drain
                    # as fillers through B(3) and the tail, so the next rep
                    # starts with vaug ready and ScalarE barely idles.
                    nxt_vaugs = alloc_vaug()
                    for tt in range(TT):
                        filler_q.append(gen_v_group(nxt_xTs, nxt_vaugs, tt))
                    nxt = (nxt_xTs, nxt_vaugs)
                if not last:
                    if kt == 0:
                        avs = (psAV.tile([DH + 1, 512], f32, tag="av", name="av0"),
                               psAV.tile([DH + 1, 512], f32, tag="av", name="av1"))
                    pending = (avs, pair, kt, es)
                    if kt == TT - 1:
                        ep_pending = (pair, tq, avs)
            filler_step(1 << 30)
            dma = nc.sync.dma_start(
                y_ap.rearrange("(t p) e -> p t e", p=P), ysb_full[:])
            if "y" in y_prev_dma:
                add_dep_helper(dma.ins, y_prev_dma["y"].ins,
                               reason="y write order across reps")
            y_prev_dma["y"] = dma
            return nxt

        if loop:
            with tc.For_i(0, rep, 1):
                emit_body(emit_xt(), alloc_vaug(), True, False)
        else:
            xTs, vaugs = emit_xt(), alloc_vaug()
            for r_i in range(rep):
                nxt = emit_body(xTs, vaugs, first=(r_i == 0),
                                has_next=(r_i + 1 < rep))
                if nxt is not None:
                    xTs, vaugs = nxt

    nc.compile()
    return nc


MMDT = "bf16"


def _get_nc(rep=1, ablate=(), mmdt=None):
    mmdt = mmdt or MMDT
    key = ("nc", rep, tuple(sorted(ablate)), mmdt)
    if key not in _cache:
        _cache[key] = _build(rep, ablate, mmdt)
    return _cache[key]


def make_in_maps(x, w_qkv, w_out, mmdt=None):
    import ml_dtypes
    mmdt = mmdt or MMDT
    dt = {"f32r": np.float32, "bf16": ml_dtypes.bfloat16,
          "fp16": np.float16, "mix": np.float16}[mmdt]
    wo_np = {"f32r": np.float32, "bf16": ml_dtypes.bfloat16,
             "fp16": np.float16, "mix": ml_dtypes.bfloat16}[mmdt]
    x = np.asarray(x, dtype=np.float32).astype(dt)
    w_qkv = np.asarray(w_qkv, dtype=np.float32).astype(dt)
    w_out = np.asarray(w_out, dtype=np.float32).astype(wo_np)
    in_maps = []
    for c in range(N_CORES):
        b, hh = divmod(c, 2)
        hsl = slice(hh * HD, (hh + 1) * HD)
        in_maps.append({
            "xT": np.ascontiguousarray(x[b].T),
            "wqT": np.ascontiguousarray(w_qkv[0 * E:1 * E][hsl].T),
            "wkT": np.ascontiguousarray(w_qkv[1 * E:2 * E][hsl].T),
            "wvT": np.ascontiguousarray(w_qkv[2 * E:3 * E][hsl].T),
            "woT": np.ascontiguousarray(w_out[:, hsl].T),
        })
    return in_maps


def combine_outputs(results):
    y = np.empty((B, NT, E), dtype=np.float32)
    for b in range(B):
        y[b] = results[2 * b]["y"] + results[2 * b + 1]["y"]
    return y


def kernel(x, w_qkv, w_out):
    from concourse.bass_utils import run_bass_kernel_spmd
    nc = _get_nc()
    in_maps = make_in_maps(x, w_qkv, w_out)
    res = run_bass_kernel_spmd(nc, in_maps, core_ids=list(range(N_CORES)))
    return combine_outputs(res.results)



# revision 13
# speedup vs baseline: 1.0910x; 1.0910x over previous
"""Multi-head attention (dense transformer block) for 8 Trainium2 NeuronCores.

Problem: x [4, 2048, 1024] f32, w_qkv [3072, 1024], w_out [1024, 1024]
  qkv = x @ w_qkv.T ; split q,k,v ; 16 heads x 64 dims
  out = softmax(q k^T / 8) v ; y = out @ w_out.T

Sharding: 8 shards = (batch b in 0..3) x (head-half hh in 0..1).
Each core handles one batch and 8 heads end-to-end: QKV projection
column-split, attention for its 8 heads, out-projection row-split ->
partial y. Host sums the two partial y's per batch. No collectives.

Kernel structure: one flat software pipeline over (pair, tq, kt) —

    v | qk(0) | B(0)+qk(1) | B(1)+qk(2)+C(1) | B(2)+qk(3) | B(3)+C(3)

  - v: value projection -> vaug bf16 [ktok, head, 65] with a ones column.
  - qk(p): q^T,k^T [128, tok] for pair p; only kT + qT(tq0) run before
    B(0), the rest (and all of qk(p+1), C(p)) drain from a filler queue
    at ~2 matmuls per kt INSIDE the attention loop, filling the PE's
    es-wait bubbles so ScalarE's exp stream never starves.
  - B(p): attention. Scores computed transposed per head S^T[ktok, qtok]
    with the two heads PAIRED via PE row-tiling (K=64 at partitions
    0/64) into adjacent PSUM banks; one ScalarE exp ACTIVATE [128, 1024]
    per k-tile covers both heads with the 1/8 scale folded in (softmax
    max-subtraction skipped; scores are O(+-6)). AV matmuls flipped to
    out[q, d] orientation: per (head, 128q-subtile, kt) one matmul with
    stationary es[128k, 128q] and moving vaug [128k, 65] — the streamed
    free dim is 65 instead of 512, halving AV's PE cost (matmul cost =
    moving-free-size x cycles/row, independent of K and M; measured
    45.8ns vs 264.5ns per matmul). The ones column makes av col 64 the
    softmax denominator per q-row. AV lags scores/exp by one k-tile
    ACROSS tq and pair boundaries (scores of kt+1 issue before the AV
    flush of kt). Softmax epilogue (emitted at the NEXT tq's first
    iteration): DVE copies the 8 denominator columns out of PSUM,
    reciprocal_approx_fast, then 8 tensor_scalar per-partition-broadcast
    multiplies normalize av into avn[128q, (j, h, d)] bf16 (PSUM freed
    after ~0.6us; no GpSimd partition_broadcast needed since q is the
    partition dim). The [q, d] -> [d, q] fix-up for the out-projection
    is 4 PE transposes [128, 128] per (pair, tq) (both heads per
    transpose land at outT partitions 0-63/64-127 directly), emitted
    through the filler queue so their DVE deps never head-of-line-block
    the PE queue, each followed by a DVE copy PSUM -> outT.
  - C(p): per-pair out-projection (K=128) into a full-size SBUF y
    accumulator (pair 1 copies, pair 3 adds); one big sync-DMA per rep
    writes y. No gpsimd DMA accum (IRAM ucode thrash vs broadcast).
  - Cross-rep pipelining (multi-rep timing builds): at B(3,tq1) the next
    rep's x DMAs fire and its value-projection groups join the filler
    queue (vaug double-buffered, v copies on DVE), so the next rep
    starts with v already materialized.
"""

import numpy as np

B = 4
NT = 2048          # tokens per batch
E = 1024           # embed dim
H = 16             # heads
DH = 64            # head dim
HD = 512           # head dims per core (8 heads)
N_CORES = 8
SCALE = DH ** -0.5
P = 128

_cache = {}


def _build(rep=1, ablate=(), mmdt="f32r", loop=False):
    import concourse.mybir as mybir
    import concourse.tile as tile
    from concourse import bacc
    from contextlib import ExitStack
    from collections import deque

    # dtype scheme: f32r/bf16/fp16 uniform; "mix" = fp16 q/k path + bf16 soft path
    f32 = mybir.dt.float32
    _qk = {"f32r": mybir.dt.float32r, "bf16": mybir.dt.bfloat16,
           "fp16": mybir.dt.float16, "mix": mybir.dt.float16}
    _soft = {"f32r": mybir.dt.bfloat16, "bf16": mybir.dt.bfloat16,
             "fp16": mybir.dt.float16, "mix": mybir.dt.bfloat16}
    f32r = _qk[mmdt]          # q/k-side matmul dtype (x, wq, wk, wv, qT, kT)
    bf16 = _soft[mmdt]        # softmax/out-side dtype (es, vaug, outT, woT)
    in_dt = {"f32r": f32, "bf16": mybir.dt.bfloat16,
             "fp16": mybir.dt.float16, "mix": mybir.dt.float16}[mmdt]
    wo_dt = {"f32r": f32, "bf16": mybir.dt.bfloat16,
             "fp16": mybir.dt.float16, "mix": mybir.dt.bfloat16}[mmdt]
    Exp = mybir.ActivationFunctionType.Exp
    Add = mybir.AluOpType.add

    nc = bacc.Bacc("TRN2", target_bir_lowering=False, debug=False,
                   enable_asserts=False, num_devices=N_CORES)

    xT_ap = nc.dram_tensor("xT", [E, NT], in_dt, kind="ExternalInput").ap()
    wqT_ap = nc.dram_tensor("wqT", [E, HD], in_dt, kind="ExternalInput").ap()
    wkT_ap = nc.dram_tensor("wkT", [E, HD], in_dt, kind="ExternalInput").ap()
    wvT_ap = nc.dram_tensor("wvT", [E, HD], in_dt, kind="ExternalInput").ap()
    woT_ap = nc.dram_tensor("woT", [HD, E], wo_dt, kind="ExternalInput").ap()
    id_ap = nc.dram_tensor("ident", [P, P], bf16, kind="ExternalInput").ap()
    y_ap = nc.dram_tensor("y", [NT, E], f32, kind="ExternalOutput").ap()

    KE = E // P        # 8 contraction tiles over embed
    MQ = HD // P       # 4 partition tiles over head dims = head pairs
    TQ = NT // 512     # 4 query chunks of 512
    TT = NT // P       # 16 token tiles of 128

    from concourse.tile_rust import add_dep_helper

    with tile.TileContext(nc) as tc, ExitStack() as ctx:
        per = ctx.enter_context(tc.tile_pool(name="per", bufs=1))
        qk_pool = ctx.enter_context(tc.tile_pool(name="qk", bufs=2))
        outT_pool = ctx.enter_context(tc.tile_pool(name="ot", bufs=2))
        es_pool = ctx.enter_context(tc.tile_pool(name="es", bufs=3))
        nrm_pool = ctx.enter_context(tc.tile_pool(name="nrm", bufs=2))
        avn_pool = ctx.enter_context(tc.tile_pool(name="avn", bufs=2))
        xT_pool = ctx.enter_context(tc.tile_pool(name="xTp", bufs=1))
        vaug_pool = ctx.enter_context(tc.tile_pool(name="vap", bufs=2))
        psS = ctx.enter_context(tc.tile_pool(name="psS", bufs=2, space="PSUM"))
        psAV = ctx.enter_context(tc.tile_pool(name="psAV", bufs=2, space="PSUM"))
        psM = ctx.enter_context(tc.tile_pool(name="psM", bufs=2, space="PSUM"))

        # rep-invariant weights (wv first: the value projection runs first)
        wv = per.tile([P, KE, HD], f32r, tag="wv")
        nc.scalar.dma_start(wv[:], wvT_ap.rearrange("(o p) m -> p o m", p=P).bitcast(f32r))
        wq = per.tile([P, KE, HD], f32r, tag="wq")
        nc.scalar.dma_start(wq[:], wqT_ap.rearrange("(o p) m -> p o m", p=P).bitcast(f32r))
        wk = per.tile([P, KE, HD], f32r, tag="wk")
        nc.scalar.dma_start(wk[:], wkT_ap.rearrange("(o p) m -> p o m", p=P).bitcast(f32r))
        woT = per.tile([P, MQ, E], bf16, tag="woT")
        nc.scalar.dma_start(woT[:], woT_ap.rearrange("(o p) e -> p o e", p=P).bitcast(bf16))
        ident = per.tile([P, P], bf16, tag="ident")
        nc.scalar.dma_start(ident[:], id_ap.bitcast(bf16))
        # full y accumulator in SBUF [tok%128, tok//128, embed]: pair 1
        # writes, pair 3 adds, one big DMA per rep writes it out. Avoids
        # gpsimd-triggered DRAM accum DMAs (IRAM ucode thrash against
        # partition_broadcast) and halves y DRAM traffic.
        ysb_full = per.tile([P, TT, E], f32, tag="ysbf")

        # Tile does not order DMAs by DRAM range: chain each y region's
        # write/accumulate DMAs explicitly (across pairs and reps).
        y_prev_dma = {}

        def emit_xt():
            xTs = []
            xT_src = xT_ap.rearrange("(o p) t -> p o t", p=P).bitcast(f32r)
            for ke in range(KE):
                xk = xT_pool.tile([P, NT], f32r, tag=f"xT{ke}", name=f"xT{ke}")
                nc.sync.dma_start(xk[:], xT_src[:, ke, :])
                xTs.append(xk)
            return xTs

        def alloc_vaug():
            # double-buffered (bufs=2) so the next rep's value projection can
            # fill fresh buffers while this rep's B(3) still reads the old
            vaug_g = [vaug_pool.tile([P, 4, 8, DH + 1], bf16, tag=f"vaug{g}",
                                     name=f"vaug{g}") for g in range(TT // 4)]
            for g in range(TT // 4):
                nc.vector.memset(vaug_g[g][:, :, :, DH:DH + 1], 1.0)
            return [vaug_g[tt // 4][:, tt % 4] for tt in range(TT)]

        def emit_body(xTs, vaugs, first, has_next):
            # PE filler queue: qk-projection and out-projection matmuls are
            # interleaved into the attention kt loop (~2 matmuls per kt) so
            # ScalarE's exp stream never waits behind a burst of projection
            # work, and the PE's es-wait bubbles get filled.
            filler_q = deque()

            def filler_step(n):
                while n > 0 and filler_q:
                    try:
                        next(filler_q[0])
                        n -= 1
                    except StopIteration:
                        filler_q.popleft()

            def gen_qk_group(mq, dst, w, tq, rot=0):
                ps = psM.tile([P, 512], f32, tag="m")
                for i in range(KE):
                    ke = (i + rot) % KE
                    nc.tensor.matmul(ps[:], w[:, ke, mq * P:(mq + 1) * P],
                                     xTs[ke][:, tq * 512:(tq + 1) * 512],
                                     start=(i == 0), stop=(i == KE - 1))
                    if i < KE - 1:
                        yield
                nc.vector.tensor_copy(dst[:, tq * 512:(tq + 1) * 512], ps[:])
                yield

            def emit_qk_group(mq, dst, w, tq, rot=0):
                for _ in gen_qk_group(mq, dst, w, tq, rot):
                    pass

            def gen_outproj(pair, outT_a, outT_b, tq):
                # y[tq tokens] (+)= outT.T @ woT for this pair of head-pairs
                for tt in range(tq * 4, tq * 4 + 4):
                    for ec in range(E // 512):
                        esl = slice(ec * 512, (ec + 1) * 512)
                        ps = psM.tile([P, 512], f32, tag="m")
                        nc.tensor.matmul(ps[:], outT_a[:, tt * P:(tt + 1) * P],
                                         woT[:, pair - 1, esl],
                                         start=True, stop=False)
                        yield
                        nc.tensor.matmul(ps[:], outT_b[:, tt * P:(tt + 1) * P],
                                         woT[:, pair, esl],
                                         start=False, stop=True)
                        if pair == 1:
                            nc.vector.tensor_copy(ysb_full[:, tt, esl], ps[:])
                        else:
                            nc.vector.tensor_tensor(ysb_full[:, tt, esl], ps[:],
                                                    ysb_full[:, tt, esl], Add)
                        yield

            def alloc_qk(mq):
                qT = qk_pool.tile([P, NT], f32r, tag="qTp", name=f"qT{mq}")
                kT = qk_pool.tile([P, NT], f32r, tag="kTp", name=f"kT{mq}")
                return qT, kT

            def qk_groups(mq, qT, kT):
                for dst, w in ((kT, wk), (qT, wq)):
                    for tq in range(TQ):
                        yield (mq, dst, w, tq)

            def gen_v_group(vxTs, vvaugs, tt):
                ps = psM.tile([P, HD], f32, tag="m")
                for i in range(KE):
                    ke = (i + tt) % KE
                    nc.tensor.matmul(ps[:], vxTs[ke][:, tt * P:(tt + 1) * P],
                                     wv[:, ke, :], start=(i == 0),
                                     stop=(i == KE - 1))
                    if i < KE - 1:
                        yield
                # copy on Vector, not Scalar: as a cross-rep filler this
                # lands in the middle of B(3)'s exp stream otherwise
                nc.vector.tensor_copy(vvaugs[tt][:, :, 0:DH],
                                      ps[:].rearrange("p (h d) -> p h d", h=8))
                yield

            def gen_eptrans(avn, outT, tq):
                # [q, d] -> [d, q] via PE transpose, one [128, 128] per
                # q-subtile covering BOTH heads (avn free layout (h, d)
                # becomes outT partitions 0-63 / 64-127). Rides the filler
                # queue with a leading yield so the transposes hit the PE
                # queue only after the DVE normalize has surely drained —
                # a PE instruction waiting on DVE would head-of-line-block
                # the next scores matmuls.
                yield
                for j in range(4):
                    pst = psM.tile([P, P], bf16, tag="m", name=f"pst{j}")
                    nc.tensor.transpose(pst[:], avn[:, j, :, :], ident[:])
                    yield
                    nc.vector.tensor_copy(
                        outT[:, tq * 512 + j * P:tq * 512 + (j + 1) * P],
                        pst[:])

            def emit_epilogue(pair, tq, avs, outT):
                # avs = per-head PSUM tiles [128q, 4, 128]; subtile j cols
                # 0:64 hold av for q-subtile j, col 64 the softmax
                # denominator. Stage denominators to SBUF (custom-DVE recip
                # cannot read PSUM), reciprocal, then per-partition
                # tensor_scalar multiplies normalize straight out of PSUM
                # (frees the AV bank in ~0.6us; next tq's AV lags 1 kt).
                avn = avn_pool.tile([P, 4, 2, DH], bf16, tag="avn")
                for h, avt in enumerate(avs):
                    dn = nrm_pool.tile([P, 4, 1], f32, tag="dn")
                    nc.vector.tensor_copy(dn[:], avt[:, :, DH:DH + 1])
                    rc = nrm_pool.tile([P, 4, 1], f32, tag="recip")
                    nc.vector.reciprocal(rc[:], dn[:])
                    for j in range(4):
                        nc.vector.tensor_scalar(
                            avn[:, j, h, :], avt[:, j, 0:DH], rc[:, j, :],
                            None, mybir.AluOpType.mult)
                filler_q.append(gen_eptrans(avn, outT, tq))

            # One flat software pipeline over (pair, tq, kt), with the AV
            # matmul lagging scores/exp by one k-tile ACROSS tq and pair
            # boundaries: scores(kt+1) always issue before the AV flush of
            # kt, so ScalarE's exp stream never has a boundary bubble. Each
            # tq's normalize epilogue is emitted at the next tq's first
            # iteration (right after its final AV lands). qk(p+1) and
            # C(p odd) matmuls drain from filler_q at ~2 per kt.
            #   v | qk(0) | B(0)+qk(1) | B(1)+qk(2)+C(1) | ... | B(3)+C(3)
            if first:
                for tt in range(TT):
                    for _ in gen_v_group(xTs, vaugs, tt):
                        pass
            qT, kT = alloc_qk(0)
            groups = list(qk_groups(0, qT, kT))
            # B(0,tq0) consumes kT group g from kt=4g and qT(tq0) from kt0:
            # emit kT(0,1) + qT(tq0) up front, drain the rest at 3/kt
            # during tq0.
            for gi in (0, 1, 4):
                emit_qk_group(*groups[gi], rot=gi)
            for gi in (2, 3, 5, 6, 7):
                filler_q.append(gen_qk_group(*groups[gi], rot=gi))

            nxt = None
            qks = {0: (qT, kT)}
            outTs = {}
            pending = None      # (avs, pair, kt, es)
            ep_pending = None   # (pair, tq, avs)
            avs = None
            qk_iter = iter(())
            NG = MQ * TQ * TT
            for g in range(NG + 1):
                pair, r = divmod(g, TQ * TT)
                tq, kt = divmod(r, TT)
                last = g == NG
                if not last and kt == 0:
                    if tq == 0:
                        qT, kT = qks[pair]
                        outTs[pair] = outT_pool.tile(
                            [P, NT], bf16, tag="outT", name=f"outT{pair}")
                        if pair + 1 < MQ:
                            qks[pair + 1] = alloc_qk(pair + 1)
                            qk_iter = qk_groups(pair + 1, *qks[pair + 1])
                        else:
                            qk_iter = iter(())
                    for _ in range(2):
                        qg = next(qk_iter, None)
                        if qg is not None:
                            filler_q.append(gen_qk_group(*qg))
                if not last:
                    qsl = slice(tq * 512, (tq + 1) * 512)
                    ksl = slice(kt * P, (kt + 1) * P)
                    sps = psS.tile([P, 2, 512], f32, tag="s")
                    nc.tensor.matmul(sps[:, 0, :], kT[0:DH, ksl],
                                     qT[0:DH, qsl], start=True, stop=True)
                    nc.tensor.matmul(sps[:, 1, :], kT[DH:P, ksl],
                                     qT[DH:P, qsl], start=True, stop=True)
                    es = es_pool.tile([P, 2, 512], bf16, tag="es")
                    nc.scalar.activation(es[:], sps[:], Exp, scale=SCALE)
                    # fillers BEFORE the av flush: the av matmul stalls on
                    # exp(kt-1)'s semaphore, and the in-order PE queue would
                    # hold the next scores (which gate exp(kt+1)) behind it.
                    # With fillers here the post-stall path to the next
                    # scores is just the av matmuls. Extra drain when the
                    # queue runs long so nothing is left for the tail.
                    n_fill = 3 if g < TT else 2
                    if len(filler_q) > 6:
                        n_fill += 1
                    filler_step(n_fill)
                if pending is not None:
                    # One accumulation group per PSUM bank (= one av tile):
                    # start/stop zero-regions are bank-granular, so only the
                    # first subtile's kt0 matmul starts the group (the bank's
                    # pending-zero makes every subtile's first write a
                    # replace) and only the last subtile's kt15 matmul stops.
                    pavs, ppair, pkt, pes = pending
                    for h in (0, 1):
                        for j in range(4):
                            nc.tensor.matmul(
                                pavs[h][:, j, 0:DH + 1],
                                pes[:, h, j * P:(j + 1) * P],
                                vaugs[pkt][:, 2 * ppair + h, :],
                                start=(pkt == 0 and j == 0),
                                stop=(pkt == TT - 1 and j == 3))
                    pending = None
                if kt == 0 and ep_pending is not None:
                    ep_pair, ep_tq, ep_avs = ep_pending
                    emit_epilogue(ep_pair, ep_tq, ep_avs, outTs[ep_pair])
                    ep_pending = None
                    if ep_pair % 2 == 1:
                        filler_q.append(gen_outproj(
                            ep_pair, outTs[ep_pair - 1], outTs[ep_pair], ep_tq))
                if kt == 0 and tq == 0 and pair == MQ - 1 and has_next:
                    # cross-rep pipeline, stage 1: the next rep's x DMAs
                    # fire at B(3,tq0) (their xT WAR deps -- qk(3) reads --
                    # resolved during B(2)). The 8 chunk DMAs serialize at
                    # ~3us each on the sync queue, so they need a full tq
                    # of lead before the v fillers start consuming them.
                    nxt_xTs = emit_xt()
                if kt == 0 and tq == 1 and pair == MQ - 1 and has_next:
                    # stage 2: the next rep's value-projection groups drain
                    # as fillers through B(3) and the tail, so the next rep
                    # starts with vaug ready and ScalarE barely idles.
                    nxt_vaugs = alloc_vaug()
                    for tt in range(TT):
                        filler_q.append(gen_v_group(nxt_xTs, nxt_vaugs, tt))
                    nxt = (nxt_xTs, nxt_vaugs)
                if not last:
                    if kt == 0:
                        avs = (psAV.tile([P, 4, P], f32, tag="av", name="av0"),
                               psAV.tile([P, 4, P], f32, tag="av", name="av1"))
                    pending = (avs, pair, kt, es)
                    if kt == TT - 1:
                        ep_pending = (pair, tq, avs)
            filler_step(1 << 30)
            dma = nc.sync.dma_start(
                y_ap.rearrange("(t p) e -> p t e", p=P), ysb_full[:])
            if "y" in y_prev_dma:
                add_dep_helper(dma.ins, y_prev_dma["y"].ins,
                               reason="y write order across reps")
            y_prev_dma["y"] = dma
            return nxt

        if loop:
            with tc.For_i(0, rep, 1):
                emit_body(emit_xt(), alloc_vaug(), True, False)
        else:
            xTs, vaugs = emit_xt(), alloc_vaug()
            for r_i in range(rep):
                nxt = emit_body(xTs, vaugs, first=(r_i == 0),
                                has_next=(r_i + 1 < rep))
                if nxt is not None:
                    xTs, vaugs = nxt

    nc.compile()
    return nc


MMDT = "bf16"


def _get_nc(rep=1, ablate=(), mmdt=None):
    mmdt = mmdt or MMDT
    key = ("nc", rep, tuple(sorted(ablate)), mmdt)
    if key not in _cache:
        _cache[key] = _build(rep, ablate, mmdt)
    return _cache[key]


def make_in_maps(x, w_qkv, w_out, mmdt=None):
    import ml_dtypes
    mmdt = mmdt or MMDT
    dt = {"f32r": np.float32, "bf16": ml_dtypes.bfloat16,
          "fp16": np.float16, "mix": np.float16}[mmdt]
    wo_np = {"f32r": np.float32, "bf16": ml_dtypes.bfloat16,
             "fp16": np.float16, "mix": ml_dtypes.bfloat16}[mmdt]
    soft_np = {"f32r": ml_dtypes.bfloat16, "bf16": ml_dtypes.bfloat16,
               "fp16": np.float16, "mix": ml_dtypes.bfloat16}[mmdt]
    x = np.asarray(x, dtype=np.float32).astype(dt)
    w_qkv = np.asarray(w_qkv, dtype=np.float32).astype(dt)
    w_out = np.asarray(w_out, dtype=np.float32).astype(wo_np)
    ident = np.eye(P, dtype=soft_np)
    in_maps = []
    for c in range(N_CORES):
        b, hh = divmod(c, 2)
        hsl = slice(hh * HD, (hh + 1) * HD)
        in_maps.append({
            "xT": np.ascontiguousarray(x[b].T),
            "wqT": np.ascontiguousarray(w_qkv[0 * E:1 * E][hsl].T),
            "wkT": np.ascontiguousarray(w_qkv[1 * E:2 * E][hsl].T),
            "wvT": np.ascontiguousarray(w_qkv[2 * E:3 * E][hsl].T),
            "woT": np.ascontiguousarray(w_out[:, hsl].T),
            "ident": ident,
        })
    return in_maps


def combine_outputs(results):
    y = np.empty((B, NT, E), dtype=np.float32)
    for b in range(B):
        y[b] = results[2 * b]["y"] + results[2 * b + 1]["y"]
    return y


def kernel(x, w_qkv, w_out):
    from concourse.bass_utils import run_bass_kernel_spmd
    nc = _get_nc()
    in_maps = make_in_maps(x, w_qkv, w_out)
    res = run_bass_kernel_spmd(nc, in_maps, core_ids=list(range(N_CORES)))
    return combine_outputs(res.results)

